# revision 1
# baseline (speedup 1.0000x reference)
"""BiMamba block (bidirectional Mamba-1 + residual + LayerNorm) on 8 TRN2
NeuronCores.

Sharding: data-parallel over batch (B=8 -> one batch element per core).
Each core runs both directions for its batch element; no collectives.

v2 engine split (channel-major, [channel, time] layouts):
  PE:    in-proj, depthwise conv (diag matmuls), x/dt-proj, state-reduce
         (identity-accumulate over the 16 SSM states), out-proj, combine.
  ACT:   Silu (conv + z gate), softplus via Exp+Ln (dt), the
         a = exp(dt*A) volume (Exp/Ln share one act table).
  DVE:   selective-scan (tensor_tensor_scan; the only engine with the op),
         most w = v*B multiplies, gating/evict fusions.
  GPSIMD: all hC = h*C multiplies + a slice of the w multiplies
         (tensor_tensor at ~3.6x DVE cost, but otherwise idle).
Emission order A_f -> A_b -> S_f -> S_b keeps every engine queue fed:
stage-A of the backward direction executes under the forward scans.

Backward direction runs the same causal pipeline on the host-reversed
sequence; the combine stage un-reverses it with an anti-identity matmul.
Final: residual + LayerNorm in fp32 (PE transposes to token-major,
bn_stats/bn_aggr), output [L, DM] per core.

The scan state itself stays fp32 inside tensor_tensor_scan; all wide
tensors are bf16 (the SSM branch contributes ~1e-3 of output magnitude).
"""

import numpy as np
import ml_dtypes
from contextlib import ExitStack

L, DM, DI, N, R, KC = 1024, 512, 1024, 16, 32, 4
P = 128
HALF = 512          # matmul moving-operand / PSUM-bank chunk (fp32 out)
NDT = DI // P       # 8 d-tiles
NTT = L // P        # 8 token tiles
NKT = DM // P       # 4 dm k-tiles

_CACHE = {}


def _emit_stage_a(nc, tc, actx, dr, sfx, xin_name):
    """Projections + conv for one direction. Returns the tensors the scan
    stage needs (persistent pool `pool` lives until the scan is done)."""
    from concourse import mybir
    AL = mybir.AluOpType
    AF = mybir.ActivationFunctionType
    F32, BF = mybir.dt.float32, mybir.dt.bfloat16

    pool = actx.enter_context(tc.tile_pool(name=f"dir_{sfx}", bufs=1))
    st = {"pool": pool}

    convb = pool.tile([P, NDT], F32, name=f"convb_{sfx}")
    nc.sync.dma_start(convb, dr[f"convb_{sfx}"])
    dtb = pool.tile([P, NDT], F32, name=f"dtb_{sfx}")
    nc.sync.dma_start(dtb, dr[f"dtb_{sfx}"])
    aneg = pool.tile([P, NDT, N], F32, name=f"aneg_{sfx}")
    nc.sync.dma_start(aneg, dr[f"aneg_{sfx}"])
    dvec = pool.tile([P, NDT], F32, name=f"dvec_{sfx}")
    nc.sync.dma_start(dvec, dr[f"dvec_{sfx}"])
    st["aneg"], st["dvec"] = aneg, dvec

    sz_dram = dr[f"szscratch_{sfx}"]
    xc_dram = dr[f"xcscratch_{sfx}"]

    wdt = pool.tile([R, DI], BF, name=f"wdt_{sfx}")
    nc.sync.dma_start(wdt, dr[f"wdt_{sfx}"])
    st["wdt"] = wdt

    with ExitStack() as sctx:
        apool = sctx.enter_context(tc.tile_pool(name=f"stgA_{sfx}", bufs=1))
        atmp = sctx.enter_context(tc.tile_pool(name=f"stgAt_{sfx}", bufs=3))
        psA = sctx.enter_context(
            tc.tile_pool(name=f"psA_{sfx}", bufs=2, space="PSUM"))

        convdiag = apool.tile([P, NDT * KC * P], BF, name=f"convdiag_{sfx}")
        nc.sync.dma_start(convdiag, dr[f"convdiag_{sfx}"])
        wx = apool.tile([P, NDT, R + 2 * N], BF, name=f"wx_{sfx}")
        nc.sync.dma_start(wx, dr[f"wx_{sfx}"])
        xT = apool.tile([P, NKT, L], BF, name=f"xT_{sfx}")
        nc.sync.dma_start(xT, dr[xin_name])

        # in-projection xz = x @ Win in two weight halves (xi then z);
        # conv runs per-channel right after its in-proj.
        xc_sb = []
        winh = apool.tile([P, NKT, DI], BF, tag="win", name=f"win_{sfx}")
        nc.sync.dma_start(winh, dr[f"win_{sfx}"][:, :, 0:DI])
        for ch in range(NDT):
            ps = psA.tile([P, L], F32, tag="psA", name="psxz")
            for h in range(2):
                for kt in range(NKT):
                    nc.tensor.matmul(
                        ps[:, h * HALF:(h + 1) * HALF],
                        lhsT=winh[:, kt, ch * P:(ch + 1) * P],
                        rhs=xT[:, kt, h * HALF:(h + 1) * HALF],
                        start=(kt == 0), stop=(kt == NKT - 1))
            xi = atmp.tile([P, L + KC - 1], BF, tag="xi", name="xi")
            nc.vector.memset(xi[:, 0:KC - 1], 0.0)
            nc.scalar.activation(xi[:, KC - 1:], ps, AF.Copy)
            # depthwise causal conv (4 taps) on the PE via diagonal tap
            # matrices over shifted windows.
            cps = psA.tile([P, L], F32, tag="psA", name="pscv")
            for h in range(2):
                for k in range(KC):
                    nc.tensor.matmul(
                        cps[:, h * HALF:(h + 1) * HALF],
                        lhsT=convdiag[:, (ch * KC + k) * P:
                                      (ch * KC + k + 1) * P],
                        rhs=xi[:, k + h * HALF:k + (h + 1) * HALF],
                        start=(k == 0), stop=(k == KC - 1))
            t = apool.tile([P, L], BF, tag=f"xc{ch}", name=f"xc{ch}")
            nc.scalar.activation(t, cps, AF.Silu,
                                 bias=convb[:, ch:ch + 1])
            xc_sb.append(t)
        winh2 = apool.tile([P, NKT, DI], BF, tag="win", name=f"win2_{sfx}")
        nc.sync.dma_start(winh2, dr[f"win_{sfx}"][:, :, DI:2 * DI])
        for ch in range(NDT):
            ps = psA.tile([P, L], F32, tag="psA", name="psz")
            for h in range(2):
                for kt in range(NKT):
                    nc.tensor.matmul(
                        ps[:, h * HALF:(h + 1) * HALF],
                        lhsT=winh2[:, kt, ch * P:(ch + 1) * P],
                        rhs=xT[:, kt, h * HALF:(h + 1) * HALF],
                        start=(kt == 0), stop=(kt == NKT - 1))
            t = atmp.tile([P, L], BF, tag="sz", name="sz")
            nc.scalar.activation(t, ps, AF.Silu)
            nc.sync.dma_start(sz_dram[ch], t)

        # x-projection: dbc = xc @ Wx  [R+2N, L] channel-major; keep the
        # SBUF copy persistent (dt is re-derived from it per scan block)
        # and park a DRAM copy for the B/C broadcast DMAs.
        dbc_ps = psA.tile([R + 2 * N, L], F32, tag="psA", name="psdbc")
        for h in range(2):
            for kt in range(NDT):
                nc.tensor.matmul(
                    dbc_ps[:, h * HALF:(h + 1) * HALF],
                    lhsT=wx[:, kt, :],
                    rhs=xc_sb[kt][:, h * HALF:(h + 1) * HALF],
                    start=(kt == 0), stop=(kt == NDT - 1))
        dbc = pool.tile([R + 2 * N, L], BF, name=f"dbc_{sfx}")
        nc.scalar.activation(dbc, dbc_ps, AF.Copy)
        nc.sync.dma_start(dr[f"dbcscratch_{sfx}"], dbc)
        st["dbc"] = dbc

        # park xc to DRAM (re-read at scan time for v = dt*xc + evict)
        for d in range(NDT):
            nc.sync.dma_start(xc_dram[d], xc_sb[d])

    st["dtb"] = dtb
    return st


def _emit_reps(nc, tc, rctx, dr, sfx, st):
    """B/C broadcast super-tiles [P, 4, L]: 4 states per tile via DMA
    partition-broadcast from the DRAM copy of dbc."""
    from concourse import mybir
    import concourse.bass as bass
    BF = mybir.dt.bfloat16

    rpool = rctx.enter_context(tc.tile_pool(name=f"reps_{sfx}", bufs=1))
    brep4, crep4 = [], []
    dbcd = dr[f"dbcscratch_{sfx}"]
    for gi, lst in ((0, brep4), (1, crep4)):
        for g in range(N // 4):
            t = rpool.tile([P, 4, L], BF, name=f"rep{gi}_{g}")
            for j in range(4):
                row = dbcd[R + gi * N + g * 4 + j:
                           R + gi * N + g * 4 + j + 1, :]
                nc.sync.dma_start(out=t[:, j, :], in_=bass.AP(
                    tensor=row.tensor, offset=row.offset,
                    ap=[[0, P]] + row.ap[1:]))
            lst.append(t)
    st.update(brep4=brep4, crep4=crep4)


def _emit_scan(nc, tc, dctx, dr, sfx, st, ibf, onep):
    """Selective scan + gating + out-projection for one direction.
    DVE runs the scans + most w = v*B; GPSIMD runs all hC = h*C and a
    slice of the w's; PE accumulates the state-reduce."""
    from concourse import mybir
    import concourse.bass as bass
    AL = mybir.AluOpType
    AF = mybir.ActivationFunctionType
    F32, BF = mybir.dt.float32, mybir.dt.bfloat16

    brep4, crep4 = st["brep4"], st["crep4"]
    aneg, dvec = st["aneg"], st["dvec"]
    wdt, dbc, dtb = st["wdt"], st["dbc"], st["dtb"]

    def rep4(ap2d):
        return bass.AP(tensor=ap2d.tensor, offset=ap2d.offset,
                       ap=[ap2d.ap[0], [0, 4]] + ap2d.ap[1:])

    yg_sb = []
    tmp2 = dctx.enter_context(tc.tile_pool(name=f"tmp_{sfx}", bufs=2))
    scanp = dctx.enter_context(tc.tile_pool(name=f"scan_{sfx}", bufs=3))
    psY = dctx.enter_context(
        tc.tile_pool(name=f"psY_{sfx}", bufs=2, space="PSUM"))
    for d in range(NDT):
        # dt = softplus(dbc[:R] @ Wdt + dtb), v = dt*xc (lazy, per block)
        dtps = psY.tile([P, L], F32, tag="psdt", name="psdt", bufs=1)
        for h in range(2):
            nc.tensor.matmul(
                dtps[:, h * HALF:(h + 1) * HALF],
                lhsT=wdt[:, d * P:(d + 1) * P],
                rhs=dbc[0:R, h * HALF:(h + 1) * HALF],
                start=True, stop=True)
        # softplus(x) = ln(1 + e^x): Exp and Ln share one act table with
        # the a = exp(dt*A) volume, so the scan phase needs no table loads.
        dtmid = tmp2.tile([P, L], BF, tag="dtm", name="dtm", bufs=2)
        nc.scalar.activation(dtmid, dtps, AF.Exp, bias=dtb[:, d:d + 1])
        dt_d = tmp2.tile([P, L], BF, tag="dtd", name="dtd", bufs=2)
        nc.scalar.activation(dt_d, dtmid, AF.Ln, bias=onep)
        xcr = tmp2.tile([P, L], BF, tag="xcr", name="xcr")
        nc.sync.dma_start(xcr, dr[f"xcscratch_{sfx}"][d])
        v_d = tmp2.tile([P, L], BF, tag="vd", name="vd", bufs=2)
        nc.vector.tensor_tensor(v_d, dt_d, xcr, AL.mult)
        yps = psY.tile([P, L], F32, tag="psY", name="psy")
        for g in range(N // 4):
            w4 = scanp.tile([P, 4, L], BF, tag="w4", name="w4", bufs=2)
            nc.vector.tensor_tensor(w4, rep4(v_d), brep4[g], AL.mult)
            # a = exp(dt*A_n); a=0 at each state's t=0 resets the carried
            # state exactly (h = 0*h_prev + w[0]).
            a4 = scanp.tile([P, 4, L], BF, tag="a4", name="a4", bufs=2)
            for j in range(4):
                n = g * 4 + j
                nc.scalar.activation(
                    a4[:, j, :], dt_d, AF.Exp,
                    scale=aneg[:, d, n:n + 1])
            nc.vector.memset(a4[:, 1:4, 0:1], 0.0)
            h4 = scanp.tile([P, 4, L], BF, tag="h4", name="h4", bufs=2)
            nc.vector.tensor_tensor_scan(
                h4.rearrange("p a b -> p (a b)"),
                a4.rearrange("p a b -> p (a b)"),
                w4.rearrange("p a b -> p (a b)"), 0.0, AL.mult, AL.add)
            hc4 = scanp.tile([P, 4, L], BF, tag="hc4", name="hc4", bufs=2)
            nc.vector.tensor_tensor(hc4, h4, crep4[g], AL.mult)
            for j in range(4):
                for h in range(2):
                    nc.tensor.matmul(
                        yps[:, h * HALF:(h + 1) * HALF],
                        lhsT=ibf,
                        rhs=hc4[:, j, h * HALF:(h + 1) * HALF],
                        start=(g == 0 and j == 0),
                        stop=(g == N // 4 - 1 and j == 3))
        # evict: yg = (y + xc*D) * silu(z)  (xcr still live from v build)
        szr = tmp2.tile([P, L], BF, tag="szr", name="szr")
        nc.sync.dma_start(szr, dr[f"szscratch_{sfx}"][d])
        yd = tmp2.tile([P, L], BF, tag="yd", name="yd")
        nc.vector.scalar_tensor_tensor(
            yd, xcr, dvec[:, d:d + 1], yps, AL.mult, AL.add)
        t = tmp2.tile([P, L], BF, tag=f"yg{d}", name=f"yg{d}", bufs=1)
        nc.vector.tensor_tensor(t, yd, szr, AL.mult)
        yg_sb.append(t)

    # out-projection: ydm = yg @ Wout  [DM, L] dm-major, f32 -> DRAM
    wout = tmp2.tile([P, NDT, DM], BF, name=f"wout_{sfx}", bufs=1)
    nc.sync.dma_start(wout, dr[f"wout_{sfx}"])
    for mt in range(NKT):
        ps = psY.tile([P, L], F32, tag="psY", name="psydm")
        for h in range(2):
            for kt in range(NDT):
                nc.tensor.matmul(
                    ps[:, h * HALF:(h + 1) * HALF],
                    lhsT=wout[:, kt, mt * P:(mt + 1) * P],
                    rhs=yg_sb[kt][:, h * HALF:(h + 1) * HALF],
                    start=(kt == 0), stop=(kt == NDT - 1))
        t = tmp2.tile([P, L], F32, tag="ydmout", name="ydmout", bufs=1)
        nc.scalar.activation(t, ps, AF.Copy)
        nc.sync.dma_start(dr[f"ydmscratch_{sfx}"][mt], t)


def _build(ln_trivial=False):
    """Build + compile the per-core Bass program (identical on all cores)."""
    import concourse.bass as bass  # noqa: F401
    import concourse.tile as tile
    from concourse import bacc, mybir

    AL = mybir.AluOpType
    AF = mybir.ActivationFunctionType
    F32, BF = mybir.dt.float32, mybir.dt.bfloat16

    nc = bacc.Bacc("TRN2", target_bir_lowering=False, debug=False,
                   num_devices=8)

    dr = {}

    def din(name, shape, dt):
        dr[name] = nc.dram_tensor(name, shape, dt, kind="ExternalInput").ap()

    din("xT", [P, NKT, L], BF)
    din("xrevT", [P, NKT, L], BF)
    din("xtok", [P, NTT, DM], F32)
    for s in ("f", "b"):
        din(f"win_{s}", [P, NKT, 2 * DI], BF)
        din(f"convdiag_{s}", [P, NDT * KC * P], BF)
        din(f"convb_{s}", [P, NDT], F32)
        din(f"wx_{s}", [P, NDT, R + 2 * N], BF)
        din(f"wdt_{s}", [R, DI], BF)
        din(f"dtb_{s}", [P, NDT], F32)
        din(f"aneg_{s}", [P, NDT, N], F32)
        din(f"dvec_{s}", [P, NDT], F32)
        din(f"wout_{s}", [P, NDT, DM], BF)
        dr[f"szscratch_{s}"] = nc.dram_tensor(
            f"szscratch_{s}", [NDT, P, L], BF, kind="Internal").ap()
        dr[f"xcscratch_{s}"] = nc.dram_tensor(
            f"xcscratch_{s}", [NDT, P, L], BF, kind="Internal").ap()
        dr[f"dbcscratch_{s}"] = nc.dram_tensor(
            f"dbcscratch_{s}", [R + 2 * N, L], BF, kind="Internal").ap()
        dr[f"ydmscratch_{s}"] = nc.dram_tensor(
            f"ydmscratch_{s}", [NKT, P, L], mybir.dt.float32,
            kind="Internal").ap()
    din("lng", [1, DM], F32)
    din("lnb", [1, DM], F32)
    din("ident32", [P, P], F32)
    din("identbf", [P, P], BF)
    din("jmat", [P, P], F32)
    out_d = nc.dram_tensor("out", [L, DM], F32, kind="ExternalOutput").ap()

    with tile.TileContext(nc) as tc, ExitStack() as octx:
        consts = octx.enter_context(tc.tile_pool(name="consts", bufs=1))
        i32 = consts.tile([P, P], F32)
        nc.sync.dma_start(i32, dr["ident32"])
        ibf = consts.tile([P, P], BF)
        nc.sync.dma_start(ibf, dr["identbf"])
        jm = consts.tile([P, P], F32)
        nc.sync.dma_start(jm, dr["jmat"])
        gbc = consts.tile([P, DM], F32)
        lng = dr["lng"]
        nc.gpsimd.dma_start(out=gbc, in_=bass.AP(
            tensor=lng.tensor, offset=lng.offset,
            ap=[[0, P]] + lng.ap[1:]))
        bbc = consts.tile([P, DM], F32)
        lnb = dr["lnb"]
        nc.gpsimd.dma_start(out=bbc, in_=bass.AP(
            tensor=lnb.tensor, offset=lnb.offset,
            ap=[[0, P]] + lnb.ap[1:]))
        epst = consts.tile([P, 1], F32)
        nc.vector.memset(epst, 1e-5)
        onep = consts.tile([P, 1], F32)
        nc.vector.memset(onep, 1.0)

        # Emission order: A_f -> A_b -> S_f -> S_b.  Stage-A of dir b
        # executes under the forward scans; each engine's in-order queue
        # always has ready work at the phase boundary.  Pool open/close
        # is strictly LIFO (Tile requirement).
        with ExitStack() as dctx_f, ExitStack() as dctx_b:
            st_f = _emit_stage_a(nc, tc, dctx_f, dr, "f", "xT")
            st_b = _emit_stage_a(nc, tc, dctx_b, dr, "b", "xrevT")
            for sfx, stx in (("f", st_f), ("b", st_b)):
                with ExitStack() as rctx:
                    _emit_reps(nc, tc, rctx, dr, sfx, stx)
                    with ExitStack() as sctx:
                        _emit_scan(nc, tc, sctx, dr, sfx, stx, ibf, onep)

        # =================== combine + LayerNorm ===================
        with ExitStack() as cctx:
            cpool = cctx.enter_context(tc.tile_pool(name="comb", bufs=2))
            spool = cctx.enter_context(tc.tile_pool(name="stats", bufs=3))
            psC = cctx.enter_context(
                tc.tile_pool(name="psC", bufs=3, space="PSUM"))
            psT = cctx.enter_context(
                tc.tile_pool(name="psT", bufs=4, space="PSUM"))
            xtok = cpool.tile([P, NTT, DM], F32, tag="xtok", bufs=1)
            nc.sync.dma_start(xtok, dr["xtok"])
            ydm = {}
            for sfx in ("f", "b"):
                ydm[sfx] = []
                for mt in range(NKT):
                    t = cpool.tile([P, L], F32, tag=f"ydm_{sfx}{mt}",
                                   name=f"ydm_{sfx}{mt}", bufs=1)
                    nc.sync.dma_start(t, dr[f"ydmscratch_{sfx}"][mt])
                    ydm[sfx].append(t)
            for tt in range(NTT):
                # transpose both directions' dm-major tiles to token-major
                yft = cpool.tile([P, DM], F32, tag="yft")
                ybr = cpool.tile([P, DM], F32, tag="ybr")
                for mt in range(NKT):
                    tp = psT.tile([P, P], F32, tag="psT")
                    nc.tensor.transpose(
                        tp, ydm["f"][mt][:, tt * P:(tt + 1) * P], i32)
                    nc.scalar.activation(
                        yft[:, mt * P:(mt + 1) * P], tp, AF.Copy)
                    tp2 = psT.tile([P, P], F32, tag="psT")
                    nc.tensor.transpose(
                        tp2, ydm["b"][mt][:, (NTT - 1 - tt) * P:
                                          (NTT - tt) * P], i32)
                    nc.scalar.activation(
                        ybr[:, mt * P:(mt + 1) * P], tp2, AF.Copy)
                # ytot = x + y_fwd + J @ y_bwd_rev  (PSUM accumulation)
                yt = psC.tile([P, DM], F32, tag="psC")
                nc.tensor.matmul(yt, lhsT=i32, rhs=xtok[:, tt, :],
                                 start=True, stop=False)
                nc.tensor.matmul(yt, lhsT=i32, rhs=yft,
                                 start=False, stop=False)
                nc.tensor.matmul(yt, lhsT=jm, rhs=ybr,
                                 start=False, stop=True)
                # LayerNorm over DM (free dim, fp32)
                stats = spool.tile([P, 6], F32, tag="bn")
                nc.vector.bn_stats(stats, yt)
                mv = spool.tile([P, 2], F32, tag="mv")
                nc.vector.bn_aggr(mv, stats)
                sd = spool.tile([P, 1], F32, tag="sd")
                nc.scalar.activation(sd, mv[:, 1:2], AF.Sqrt, bias=epst)
                rs = spool.tile([P, 1], F32, tag="rs")
                nc.vector.reciprocal(rs, sd)
                nmu = spool.tile([P, 1], F32, tag="nmu")
                nc.vector.scalar_tensor_tensor(
                    nmu, mv[:, 0:1], -1.0, rs, AL.mult, AL.mult)
                ycn = cpool.tile([P, DM], F32, tag="ycn")
                nc.scalar.activation(ycn, yt, AF.Identity,
                                     bias=nmu, scale=rs)
                if ln_trivial:
                    nc.sync.dma_start(out_d[tt * P:(tt + 1) * P, :], ycn)
                else:
                    o1 = cpool.tile([P, DM], F32, tag="o1")
                    nc.vector.tensor_tensor(o1, ycn, gbc, AL.mult)
                    o2 = cpool.tile([P, DM], F32, tag="o2")
                    nc.vector.tensor_tensor(o2, o1, bbc, AL.add)
                    nc.sync.dma_start(out_d[tt * P:(tt + 1) * P, :], o2)

    nc.compile()
    return nc


def _host_inputs(inputs):
    """Shared (per-core-independent) input arrays, SBUF-layouted."""
    bf = ml_dtypes.bfloat16
    f32 = np.float32

    def tile3(a, nk):
        # [nk*P, F] -> [P, nk, F]
        F = a.shape[-1]
        return np.ascontiguousarray(
            a.reshape(nk, P, F).transpose(1, 0, 2))

    m = {}
    for s in ("f", "b"):
        m[f"win_{s}"] = tile3(inputs[f"in_w_{s}"], NKT).astype(bf)
        cw = inputs[f"conv_w_{s}"].reshape(NDT, P, KC)
        cd = np.zeros((NDT, KC, P, P), dtype=np.float32)
        for dt_ in range(NDT):
            for k in range(KC):
                np.fill_diagonal(cd[dt_, k], cw[dt_, :, k])
        # lhsT layout: [p, (dt,k)*P + m] with diag on (p == m)
        m[f"convdiag_{s}"] = np.ascontiguousarray(
            cd.transpose(2, 0, 1, 3).reshape(P, NDT * KC * P)).astype(bf)
        m[f"convb_{s}"] = np.ascontiguousarray(
            inputs[f"conv_b_{s}"].reshape(NDT, P).T).astype(f32)
        m[f"wx_{s}"] = tile3(inputs[f"xproj_w_{s}"], NDT).astype(bf)
        m[f"wdt_{s}"] = inputs[f"dt_w_{s}"].astype(bf)
        m[f"dtb_{s}"] = np.ascontiguousarray(
            inputs[f"dt_b_{s}"].reshape(NDT, P).T).astype(f32)
        m[f"aneg_{s}"] = tile3(-np.exp(inputs[f"A_log_{s}"]), NDT).astype(f32)
        m[f"dvec_{s}"] = np.ascontiguousarray(
            inputs[f"D_{s}"].reshape(NDT, P).T).astype(f32)
        m[f"wout_{s}"] = tile3(inputs[f"out_w_{s}"], NDT).astype(bf)
    m["lng"] = inputs["ln_g"].reshape(1, DM).astype(f32)
    m["lnb"] = inputs["ln_b"].reshape(1, DM).astype(f32)
    m["ident32"] = np.eye(P, dtype=f32)
    m["identbf"] = np.eye(P).astype(bf)
    m["jmat"] = np.eye(P, dtype=f32)[::-1].copy()
    return m


def _run(inputs, trace=False, trace_kwargs=None):
    from concourse.bass_utils import run_bass_kernel_spmd

    ln_trivial = bool(
        np.all(np.asarray(inputs["ln_g"]) == 1.0)
        and np.all(np.asarray(inputs["ln_b"]) == 0.0))
    key = ("nc", ln_trivial)
    if key not in _CACHE:
        _CACHE[key] = _build(ln_trivial=ln_trivial)
    nc = _CACHE[key]

    bf = ml_dtypes.bfloat16
    x = np.asarray(inputs["x"], dtype=np.float32)          # [8, L, DM]
    shared = _host_inputs({k: np.asarray(v) for k, v in inputs.items()
                           if k != "x"})

    in_maps = []
    for c in range(8):
        xb = x[c]                                          # [L, DM]
        m = dict(shared)
        m["xT"] = np.ascontiguousarray(
            xb.T.reshape(NKT, P, L).transpose(1, 0, 2)).astype(bf)
        m["xrevT"] = np.ascontiguousarray(
            xb[::-1].T.reshape(NKT, P, L).transpose(1, 0, 2)).astype(bf)
        m["xtok"] = np.ascontiguousarray(
            xb.reshape(NTT, P, DM).transpose(1, 0, 2)).astype(np.float32)
        in_maps.append(m)

    res = run_bass_kernel_spmd(nc, in_maps, core_ids=list(range(8)),
                               trace=trace, **(trace_kwargs or {}))
    out = np.stack([res.results[c]["out"] for c in range(8)], axis=0)
    return out.astype(np.float32), res


def kernel(**inputs):
    out, _ = _run(inputs)
    return out


if __name__ == "__main__":
    rng = np.random.default_rng(0)
    fake = {"x": rng.standard_normal((8, L, DM), dtype=np.float32)}
    for s in ("f", "b"):
        fake[f"in_w_{s}"] = rng.standard_normal((DM, 2 * DI), dtype=np.float32) * 0.02
        fake[f"conv_w_{s}"] = rng.standard_normal((DI, KC), dtype=np.float32) * 0.3
        fake[f"conv_b_{s}"] = np.zeros(DI, np.float32)
        fake[f"xproj_w_{s}"] = rng.standard_normal((DI, R + 2 * N), dtype=np.float32) * 0.02
        fake[f"dt_w_{s}"] = rng.standard_normal((R, DI), dtype=np.float32) * 0.02
        fake[f"dt_b_{s}"] = rng.standard_normal(DI, dtype=np.float32) * 0.1 - 4.0
        fake[f"A_log_{s}"] = np.tile(np.log(np.arange(1, N + 1, dtype=np.float32)), (DI, 1))
        fake[f"D_{s}"] = np.ones(DI, np.float32)
        fake[f"out_w_{s}"] = rng.standard_normal((DI, DM), dtype=np.float32) * 0.02
    fake["ln_g"] = np.ones(DM, np.float32)
    fake["ln_b"] = np.zeros(DM, np.float32)
    o = kernel(**fake)
    print("out", o.shape, o.dtype, float(np.abs(o).max()))

